# revision 1
# baseline (speedup 1.0000x reference)
"""ChunkedDiagonalMLP Trainium2 kernel — 8-core SPMD, data-parallel over tokens.

Math (per token row x of width 4096, split into 8 chunks of 512):
    h_n  = gelu(x_n @ w1[n] + b1[n])          (exact erf gelu)
    y_n  = h_n @ w2[n] + b2[n]
    out  = LayerNorm(concat_n(y_n) + x) * ln_g + ln_b

Strategy per core (2048 tokens):
  - layer 1 feature-major: stationary w1 blocks, moving x^T (host-pre-
    transposed, bf16) -> h^T in PSUM, gelu+bias fused on ScalarE -> bf16 h^T
  - layer 2 token-major: stationary h^T blocks, moving w2 -> y in PSUM
  - residual from a bf16 token-major x read (y accumulates fp32 in PSUM;
    only x's own bf16 rounding enters the error), fused with partial mean
    accumulation (scalar_tensor_tensor accum_out)
  - sum-of-squares via Square activation accum_out on ScalarE
  - LayerNorm apply via tensor_scalar (sub mean, mul rstd), DMA out fp32
"""

import numpy as np
import ml_dtypes
from contextlib import ExitStack

import concourse.bass as bass
import concourse.mybir as mybir
import concourse.tile as tile
from concourse.bass_utils import run_bass_kernel_spmd

N_CORES = 8
D = 4096
NCH = 8          # chunks
CH = 512         # chunk width
KT = CH // 128   # k-tiles per chunk (4)
S = 2048         # tokens per core
TG = 512         # tokens per group
NG = S // TG     # 4 groups
IT = TG // 128   # 128-token tiles per group (4)
EPS = 1e-5

F32 = mybir.dt.float32
BF16 = mybir.dt.bfloat16
BF = ml_dtypes.bfloat16


def _split_excess_waits(nc, limit=1):
    """walrus CoreV3 codegen rejects instructions with too many sem waits
    (Drain allows only 1); move extras onto preceding same-engine NoOps."""
    n_split = 0
    for bb in nc.main_func.blocks:
        new_insts = []
        changed = False
        for inst in bb.instructions:
            lim = limit
            si = inst.sync_info
            if si is not None and si.on_wait and len(si.on_wait) > lim:
                waits = list(si.on_wait)
                extra, keep = waits[:-lim], waits[-lim:]
                for i in range(0, len(extra), lim):
                    nop = mybir.InstNoOp(
                        name=f"{inst.name}-ws{i}",
                        engine=inst.engine,
                        ins=[],
                        outs=[],
                        sync_info=mybir.SyncInfo(
                            on_wait=list(extra[i : i + lim]), on_update=[]
                        ),
                    )
                    new_insts.append(nop)
                    n_split += 1
                inst.sync_info = mybir.SyncInfo(
                    on_wait=list(keep), on_update=list(si.on_update)
                )
                changed = True
            new_insts.append(inst)
        if changed:
            bb.instructions[:] = new_insts
    return n_split


def _build(use_b2, use_lng, use_lnb, reps=1, pp_bufs=4, h_bufs=3, x_bufs=4):
    nc = bass.Bass()
    # x^T per core: [n, k, c(128), t] bf16
    xT_e = nc.declare_dram_parameter("xT", [NCH, KT, 128, S], BF16, isOutput=False)
    # token-major x rows (bf16) for residual
    xr_e = nc.declare_dram_parameter("xr", [S, D], BF16, isOutput=False)
    # weights: [n, c(128), k, d] bf16 (host pre-permuted so partition lines
    # are 4KB contiguous)
    w1_e = nc.declare_dram_parameter("w1", [NCH, 128, KT, CH], BF16, isOutput=False)
    w2_e = nc.declare_dram_parameter("w2", [NCH, 128, KT, CH], BF16, isOutput=False)
    # b1 rearranged to [128, n*4+j] columns
    b1_e = nc.declare_dram_parameter("b1c", [128, NCH * KT], F32, isOutput=False)
    b2_e = nc.declare_dram_parameter("b2", [NCH, CH], F32, isOutput=False)
    lng_e = nc.declare_dram_parameter("ln_g", [D], F32, isOutput=False)
    lnb_e = nc.declare_dram_parameter("ln_b", [D], F32, isOutput=False)
    out_e = nc.declare_dram_parameter("out", [S, D], F32, isOutput=True)

    with tile.TileContext(nc) as tc:
        with ExitStack() as ctx:
            opool = ctx.enter_context(tc.tile_pool(name="opool", bufs=1))
            xpool = ctx.enter_context(tc.tile_pool(name="xpool", bufs=x_bufs))
            hpool = ctx.enter_context(tc.tile_pool(name="hpool", bufs=h_bufs))
            spool = ctx.enter_context(tc.tile_pool(name="spool", bufs=2))
            cpool = ctx.enter_context(tc.tile_pool(name="cpool", bufs=1))
            pp_h = ctx.enter_context(tc.tile_pool(name="pp_h", bufs=pp_bufs, space="PSUM"))
            pp_y = ctx.enter_context(tc.tile_pool(name="pp_y", bufs=pp_bufs, space="PSUM"))

            # ---- constants / weights (resident) ----
            b1_sb = cpool.tile([128, NCH * KT], F32)
            nc.sync.dma_start(out=b1_sb, in_=b1_e[:, :])
            eps_sb = cpool.tile([128, 1], F32)
            nc.vector.memset(eps_sb, EPS)

            w1_sb = []
            w2_sb = []
            for n in range(NCH):
                w1t = cpool.tile([128, KT, CH], BF16, name=f"w1_{n}")
                nc.sync.dma_start(out=w1t, in_=w1_e[n])
                w1_sb.append(w1t)
                w2t = cpool.tile([128, KT, CH], BF16, name=f"w2_{n}")
                nc.sync.dma_start(out=w2t, in_=w2_e[n])
                w2_sb.append(w2t)

            b2_sb = None
            if use_b2:
                b2_sb = cpool.tile([128, NCH, CH], F32)
                nc.gpsimd.dma_start(
                    out=b2_sb,
                    in_=bass.AP(
                        tensor=b2_e.tensor,
                        offset=b2_e.offset,
                        ap=[[0, 128], b2_e.ap[0], b2_e.ap[1]],
                    ),
                )
            lng_sb = None
            if use_lng:
                lng_sb = cpool.tile([128, D], F32)
                nc.gpsimd.dma_start(
                    out=lng_sb,
                    in_=bass.AP(
                        tensor=lng_e.tensor, offset=lng_e.offset,
                        ap=[[0, 128], lng_e.ap[0]],
                    ),
                )
            lnb_sb = None
            if use_lnb:
                lnb_sb = cpool.tile([128, D], F32)
                nc.gpsimd.dma_start(
                    out=lnb_sb,
                    in_=bass.AP(
                        tensor=lnb_e.tensor, offset=lnb_e.offset,
                        ap=[[0, 128], lnb_e.ap[0]],
                    ),
                )

            for rep in range(reps):
              for g in range(NG):
                tsl = slice(g * TG, (g + 1) * TG)
                out_sb = opool.tile([128, IT, D], F32, name="out_sb", tag="out_sb")
                sums = spool.tile([128, IT, NCH], F32, name="sums", tag="sums")
                sqs = spool.tile([128, IT, NCH], F32, name="sqs", tag="sqs")

                for n in range(NCH):
                    # x^T slice [c=128, k, t=TG] bf16
                    xT_sb = xpool.tile([128, KT, TG], BF16, name="xT_sb")
                    nc.sync.dma_start(
                        out=xT_sb,
                        in_=xT_e[n, :, :, tsl].rearrange("k c t -> c k t"),
                    )
                    # token-major fp32 x rows for the residual: [p, i, d-slice]
                    xr_sb = xpool.tile([128, IT, CH], BF16, name="xr_sb")
                    nc.sync.dma_start(
                        out=xr_sb,
                        in_=xr_e[tsl, n * CH : (n + 1) * CH].rearrange(
                            "(i p) d -> p i d", p=128
                        ),
                    )

                    # ---- layer 1: h^T[j] = gelu(w1^T x^T + b1) ----
                    hT = hpool.tile([128, KT, TG], BF16, name="hT")
                    for j in range(KT):
                        ph = pp_h.tile([128, TG], F32, tag="ph", name="ph")
                        for k in range(KT):
                            nc.tensor.matmul(
                                ph,
                                w1_sb[n][:, k, j * 128 : (j + 1) * 128],
                                xT_sb[:, k, :],
                                start=(k == 0),
                                stop=(k == KT - 1),
                            )
                        nc.scalar.activation(
                            out=hT[:, j, :],
                            in_=ph,
                            func=mybir.ActivationFunctionType.Gelu,
                            bias=b1_sb[:, n * KT + j : n * KT + j + 1],
                        )

                    # ---- layer 2: y[i] = h^T[:,i].T w2 ( + x residual ) ----
                    for i in range(IT):
                        py = pp_y.tile([128, CH], F32, tag="py", name="py")
                        for j in range(KT):
                            nc.tensor.matmul(
                                py,
                                hT[:, j, i * 128 : (i + 1) * 128],
                                w2_sb[n][:, j, :],
                                start=(j == 0),
                                stop=(j == KT - 1),
                            )
                        osl = out_sb[:, i, n * CH : (n + 1) * CH]
                        # out = y + x ; accumulate per-token partial sum
                        nc.vector.scalar_tensor_tensor(
                            out=osl,
                            in0=py,
                            scalar=1.0,
                            in1=xr_sb[:, i, :],
                            op0=mybir.AluOpType.mult,
                            op1=mybir.AluOpType.add,
                            accum_out=sums[:, i, n : n + 1],
                        )
                        if use_b2:
                            nc.vector.tensor_add(osl, osl, b2_sb[:, n, :])
                        # sum of squares on ScalarE (scratch result discarded)
                        sq = spool.tile([128, CH], BF16, tag="sq", name="sq")
                        nc.scalar.activation(
                            out=sq,
                            in_=osl,
                            func=mybir.ActivationFunctionType.Square,
                            accum_out=sqs[:, i, n : n + 1],
                        )

                # ---- LayerNorm stats for the whole group ----
                mu = spool.tile([128, IT], F32, name="mu")
                nc.vector.tensor_reduce(
                    out=mu, in_=sums, axis=mybir.AxisListType.X, op=mybir.AluOpType.add
                )
                nc.vector.tensor_scalar_mul(out=mu, in0=mu, scalar1=1.0 / D)
                ssum = spool.tile([128, IT], F32, name="ssum")
                nc.vector.tensor_reduce(
                    out=ssum, in_=sqs, axis=mybir.AxisListType.X, op=mybir.AluOpType.add
                )
                # var = E[x^2] - mu^2 ; rstd = 1/sqrt(var + eps)
                var = spool.tile([128, IT], F32, name="var")
                nc.vector.tensor_scalar_mul(out=var, in0=ssum, scalar1=1.0 / D)
                mu2 = spool.tile([128, IT], F32, name="mu2")
                nc.vector.tensor_mul(out=mu2, in0=mu, in1=mu)
                nc.vector.tensor_sub(out=var, in0=var, in1=mu2)
                rs = spool.tile([128, IT], F32, name="rs")
                nc.scalar.activation(
                    out=rs, in_=var,
                    func=mybir.ActivationFunctionType.Sqrt,
                    bias=eps_sb,
                )
                nc.vector.reciprocal(out=rs, in_=rs)

                for i in range(IT):
                    nc.vector.tensor_scalar(
                        out=out_sb[:, i, :],
                        in0=out_sb[:, i, :],
                        scalar1=mu[:, i : i + 1],
                        scalar2=rs[:, i : i + 1],
                        op0=mybir.AluOpType.subtract,
                        op1=mybir.AluOpType.mult,
                    )
                    if use_lng:
                        nc.vector.tensor_mul(
                            out=out_sb[:, i, :], in0=out_sb[:, i, :], in1=lng_sb
                        )
                    if use_lnb:
                        nc.vector.tensor_add(
                            out=out_sb[:, i, :], in0=out_sb[:, i, :], in1=lnb_sb
                        )
                    nc.sync.dma_start(
                        out=out_e[tsl, :].rearrange("(i p) d -> p i d", p=128)[
                            :, i, :
                        ],
                        in_=out_sb[:, i, :],
                    )

    _split_excess_waits(nc)
    return nc


_CACHE = {}


def kernel(x, w1, b1, w2, b2, ln_g, ln_b):
    x = np.asarray(x)
    w1 = np.asarray(w1, dtype=np.float32)
    w2 = np.asarray(w2, dtype=np.float32)
    b1 = np.asarray(b1, dtype=np.float32)
    b2 = np.asarray(b2, dtype=np.float32)
    ln_g = np.asarray(ln_g, dtype=np.float32)
    ln_b = np.asarray(ln_b, dtype=np.float32)
    B, L, d = x.shape
    assert d == D and B * L == N_CORES * S, (x.shape,)

    use_b2 = bool(np.any(b2 != 0.0))
    use_lng = bool(np.any(ln_g != 1.0))
    use_lnb = bool(np.any(ln_b != 0.0))

    key = (use_b2, use_lng, use_lnb)
    if key not in _CACHE:
        _CACHE[key] = _build(*key)
    nc = _CACHE[key]

    # host-side input prep (sharding + layout)
    x2 = np.ascontiguousarray(x.reshape(B * L, D).astype(np.float32))
    w1h = np.ascontiguousarray(
        w1.reshape(NCH, KT, 128, CH).transpose(0, 2, 1, 3).astype(BF)
    )
    w2h = np.ascontiguousarray(
        w2.reshape(NCH, KT, 128, CH).transpose(0, 2, 1, 3).astype(BF)
    )
    b1h = np.ascontiguousarray(
        b1.reshape(NCH, KT, 128).transpose(2, 0, 1).reshape(128, NCH * KT)
    )

    in_maps = []
    for c in range(N_CORES):
        rows = x2[c * S : (c + 1) * S]  # [S, D] fp32
        xTh = np.ascontiguousarray(rows.T).astype(BF).reshape(NCH, KT, 128, S)
        in_maps.append(
            {
                "xT": xTh,
                "xr": rows.astype(BF),
                "w1": w1h,
                "w2": w2h,
                "b1c": b1h,
                "b2": b2,
                "ln_g": ln_g,
                "ln_b": ln_b,
            }
        )

    res = run_bass_kernel_spmd(nc, in_maps, list(range(N_CORES)))
    out = np.concatenate([res.results[c]["out"] for c in range(N_CORES)], axis=0)
    return out.reshape(B, L, D).astype(np.float32)

